# revision 17
# baseline (speedup 1.0000x reference)
"""Joint-entropy (KDE logsumexp over 3x3 windows) Trainium2 kernel, v5.

Math: for each 3x3 window of pixel vectors v_n (C=3 channels),
  out[i,j] = log_norm - (1/9) * sum_n log(S_n),  S_n = sum_m exp(-2*||v_n-v_m||^2)

Per-pair Gaussians via Act's Derivative_Erf: derf(sqrt(2)*d) =
(2/sqrt(pi))*exp(-2 d^2), so prod_c derf = k*exp(-2||d||^2), k=(2/sqrt(pi))^3.
Every E value carries k; the self term "+1" becomes "+k" (folded into the
same-row A combos) and the final affine adds ln(k):
  out = (log_norm + ln k) - (1/9) * ln prod_n (k + sum_{m!=n} kE_nm).

Sharding: 8 cores = 4 batches x 2 row-halves; host-prepped bf16 slab
[130, 3, 264] per core (129 rows + 1 pad row, width padded 4 each side);
output [127, 254] f32. partitions = image rows; X[p, s] = row p+s (fused
overlapping DMA reads) makes all row gaps reachable with partition-0
operands. Input lands as two DMAs (s=0,1 then s=2) so compute starts as
soon as the first two views arrive.

E classes (sub -> Derivative_Erf -> 2 channel-muls), anchor cols -2..257:
  M1 [128,5,260] a=1 (X0 vs X1),  M2 [128,5,260] a=2 (X0 vs X2; row 127
  junk from the pad row, never consumed),  ME [128,2(lo,hi),2,260] a=0.

Stage C: sliding 3-sums over b (D = down pairs, U = up pairs with
per-plane col shifts via plane-stride tricks), A combos (+k folded, merged
over lo/hi). Y = Ahi + U1 gives roles S1 = Y + D1@+1 and S2 = Y@+1 + U2.
The two partition shifts go through DRAM round-trips (SBUF->DRAM->SBUF on
the two HWDGE queues): direct SBUF->SBUF DMA measured ~6x slower than the
DRAM bounce, which fans out over all 16 DMA engines.
Stage D: 4 muls, Ln, affine; output split across both HWDGE queues
(keeps the slow SWDGE drain off the exit path).
"""

import dataclasses

import numpy as np
import ml_dtypes

import concourse.bacc as bacc
import concourse.tile as tile
from concourse import mybir
from concourse.bass_utils import run_bass_kernel_spmd

F32 = mybir.dt.float32
BF16 = mybir.dt.bfloat16
AOP = mybir.AluOpType
AF = mybir.ActivationFunctionType

B = 4
C = 3
W = 256
PAD = 4           # host zero-pad each side
WT = W + 2 * PAD  # 264
WA = W + 4        # 260: anchor cols -2..257
ROWS_IN = 129
P = 128
POUT = 127
WOUT = 254
SROW = C * WT     # one input row in elements (792)
SQRT2 = float(np.sqrt(2.0))
CDERF = float(2.0 / np.sqrt(np.pi))
K = CDERF**3
LOG_NORM = float(np.log(9.0) + 3.0 * np.log(np.sqrt(2.0 * np.pi) * 0.5))
AFFINE_C = LOG_NORM + 3.0 * float(np.log(CDERF))


def _with_dims(base_ap, dims):
    """Replace free dims of `base_ap` (partition dim kept) with the given
    [stride, count] pairs (strides in elements)."""
    ap = [list(base_ap.ap[0])] + [list(d) for d in dims]
    return dataclasses.replace(base_ap, ap=ap)


def _build_program():
    nc = bacc.Bacc("TRN2")
    xin = nc.dram_tensor("xin", (ROWS_IN + 1, C, WT), BF16, kind="ExternalInput")
    yout = nc.dram_tensor("yout", (POUT, WOUT), F32, kind="ExternalOutput")

    with tile.TileContext(nc) as tc:
        with (
            tc.tile_pool(name="p", bufs=1) as tp,
            tc.tile_pool(name="dp", bufs=1, space="DRAM") as dp,
        ):
            def ap_of(base, elem_off, dims):
                return dataclasses.replace(
                    _with_dims(base, dims), offset=base.offset + elem_off
                )

            # ---- load X[p, s] = input row p+s (s=0,1 first, then s=2) ---
            X = tp.tile([P, 3, C, WT], BF16, tag="x")
            src01 = _with_dims(xin[0:P, :, :], [[SROW, 2], [WT, C], [1, WT]])
            nc.gpsimd.dma_start(out=X[:, 0:2], in_=src01)
            src2 = ap_of(xin[0:P, :, :], 2 * SROW, [[WT, C], [1, WT]])
            nc.gpsimd.dma_start(out=X[:, 2], in_=src2)

            ME = tp.tile([P, 2, 2, WA], BF16, tag="me")  # a=0: s -> lo, hi

            def cls(tag, s_a, s_b, nb, c0, out_ap):
                """E class: anchor X[:, s_a] bcast over b; other X[:, s_b]
                at col c0+b; writes k*exp(-2 d2) planes into out_ap."""
                xa = X[0:P, 0, 0, 2 : 2 + WA]
                anchor = ap_of(xa, SROW * s_a, [[0, nb], [WT, C], [1, WA]])
                xb = X[0:P, 0, 0, c0 : c0 + WA]
                other = ap_of(xb, SROW * s_b, [[1, nb], [WT, C], [1, WA]])
                d = tp.tile([P, nb, C, WA], BF16, tag=f"d_{tag}")
                nc.vector.tensor_sub(d, anchor, other)
                g = tp.tile([P, nb, C, WA], BF16, tag=f"g_{tag}")
                nc.scalar.activation(g, d, AF.Derivative_Erf, scale=SQRT2)
                g01 = tp.tile([P, nb, WA], BF16, tag=f"g01_{tag}")
                nc.vector.tensor_mul(g01, g[:, :, 0, :], g[:, :, 1, :])
                nc.vector.tensor_mul(out_ap, g01, g[:, :, 2, :])

            def d_combo(mt, tag):
                t4 = tp.tile([P, 4, W], BF16, tag=f"t4{tag}")
                nc.vector.tensor_add(t4, mt[:, 0:4, 2 : 2 + W], mt[:, 1:5, 2 : 2 + W])
                out = tp.tile([P, 3, W], BF16, tag=f"dc{tag}")
                nc.vector.tensor_add(out, t4[:, 0:3, :], mt[:, 2:5, 2 : 2 + W])
                return out

            def u_combo(mt, tag):
                # plane t = sum_{j=t..t+2} mt[:, j, col + 4 - j]
                t4 = tp.tile([P, 4, W], BF16, tag=f"u4{tag}")
                in0 = ap_of(mt[0:P, 0, 0:W], 4, [[WA - 1, 4], [1, W]])
                in1 = ap_of(mt[0:P, 0, 0:W], WA + 3, [[WA - 1, 4], [1, W]])
                nc.vector.tensor_add(t4, in0, in1)
                out = tp.tile([P, 3, W], BF16, tag=f"uc{tag}")
                in2 = ap_of(mt[0:P, 0, 0:W], 2 * WA + 2, [[WA - 1, 3], [1, W]])
                nc.vector.tensor_add(out, t4[:, 0:3, :], in2)
                return out

            # ---- a=1 class first: its combos feed the DRAM bounce -------
            M1 = tp.tile([P, 5, WA], BF16, tag="m1")
            cls("m1", 0, 1, 5, 0, M1)
            D1 = d_combo(M1, "d1")
            U1 = u_combo(M1, "u1")

            # DRAM bounce #1: D1 rows 1..127 -> D1h.  Writes (SBUF->DRAM) fan
            # out on the SP HWDGE queue; reads (DRAM->SBUF) only fan out on
            # the gpsimd SWDGE queue, so the read-back goes there.
            d1d = dp.tile([P, 3, W], BF16, tag="d1d")
            nc.sync.dma_start(out=d1d, in_=D1)
            D1h = tp.tile([POUT, 3, W], BF16, tag="d1h")
            nc.gpsimd.dma_start(out=D1h, in_=d1d[1:P])

            # ---- a=0 classes, A combos, Y -------------------------------
            cls("e0hi", 1, 1, 2, 3, ME[:, 1])
            cls("e0lo", 0, 0, 2, 3, ME[:, 0])

            # A combos (+k), merged over (lo, hi):
            #  nc=0: E01(j)+E02(j); nc=1: E01(j-1)+E01(j); nc=2: E02(j-2)+E01(j-1)
            SE = 2 * WA  # s stride of ME

            def me_ap(k, col0):
                return ap_of(ME[0:P, 0, 0, 0:W], WA * k + col0, [[SE, 2], [1, W]])

            A = tp.tile([P, 2, 3, W], BF16, tag="a")
            pairs = [((0, 2), (1, 2)), ((0, 1), (0, 2)), ((1, 0), (0, 1))]
            for ncol, ((k0, o0), (k1, o1)) in enumerate(pairs):
                nc.vector.scalar_tensor_tensor(
                    out=A[:, :, ncol, :],
                    in0=me_ap(k0, o0),
                    scalar=K,
                    in1=me_ap(k1, o1),
                    op0=AOP.add,
                    op1=AOP.add,
                )

            Y = tp.tile([P, 3, W], BF16, tag="y")
            nc.vector.tensor_add(Y, A[:, 1, :, :], U1)

            # ---- a=2 class + combos; Y bounce on the Act queue ----------
            M2 = tp.tile([P, 5, WA], BF16, tag="m2")
            cls("m2", 0, 2, 5, 0, M2)  # row 127 junk (pad row), not consumed

            yd = dp.tile([P, 3, W], BF16, tag="yd")
            nc.sync.dma_start(out=yd, in_=Y)
            Yh = tp.tile([POUT, 3, W], BF16, tag="yh")
            nc.gpsimd.dma_start(out=Yh, in_=yd[1:P])

            D2 = d_combo(M2, "d2")
            U2 = u_combo(M2, "u2")

            # ---- role sums [127, 3(nc), 254] ----------------------------
            def diag(base, plane0, dplane):
                """[POUT, 3, WOUT] view: plane i at (plane0 + i*dplane, col+i);
                base = [POUT, WOUT] slice of a W-plane-stride tile."""
                return ap_of(base, plane0 * W, [[dplane * W + 1, 3], [1, WOUT]])

            S0 = tp.tile([POUT, 3, WOUT], BF16, tag="s0")
            nc.vector.tensor_add(
                S0, diag(A[0:POUT, 0, 0, 0:WOUT], 0, 1),
                diag(D1[0:POUT, 0, 0:WOUT], 2, -1),
            )
            nc.vector.tensor_add(S0, S0, diag(D2[0:POUT, 0, 0:WOUT], 2, -1))
            S1 = tp.tile([POUT, 3, WOUT], BF16, tag="s1")
            nc.vector.tensor_add(
                S1, diag(Y[0:POUT, 0, 0:WOUT], 0, 1),
                diag(D1h[0:POUT, 0, 0:WOUT], 2, -1),
            )
            S2 = tp.tile([POUT, 3, WOUT], BF16, tag="s2")
            nc.vector.tensor_add(
                S2, diag(Yh[0:POUT, 0, 0:WOUT], 0, 1),
                diag(U2[0:POUT, 0, 0:WOUT], 0, 1),
            )

            # ---- stage D: product, log, affine --------------------------
            T0 = tp.tile([POUT, 3, WOUT], BF16, tag="t0")
            nc.vector.tensor_mul(T0, S0, S1)
            T1 = tp.tile([POUT, 3, WOUT], BF16, tag="t1")
            nc.vector.tensor_mul(T1, T0, S2)
            R = tp.tile([POUT, WOUT], BF16, tag="r")
            nc.vector.tensor_mul(R, T1[:, 0, :], T1[:, 1, :])
            PP = tp.tile([POUT, WOUT], BF16, tag="pp")
            nc.vector.tensor_mul(PP, R, T1[:, 2, :])
            L = tp.tile([POUT, WOUT], F32, tag="lnp")
            nc.scalar.activation(L, PP, AF.Ln)
            OUT = tp.tile([POUT, WOUT], F32, tag="out")
            nc.vector.tensor_scalar(
                out=OUT,
                in0=L,
                scalar1=-1.0 / 9.0,
                scalar2=AFFINE_C,
                op0=AOP.mult,
                op1=AOP.add,
            )
            nc.sync.dma_start(out=yout[0:64, :], in_=OUT[0:64, :])
            nc.scalar.dma_start(out=yout[64:POUT, :], in_=OUT[64:POUT, :])
    if not nc.is_finalized():
        nc.finalize()
    return nc


_PROGRAM = None


def _get_program():
    global _PROGRAM
    if _PROGRAM is None:
        _PROGRAM = _build_program()
    return _PROGRAM


def _shard_inputs(x):
    x = np.asarray(x, dtype=np.float32)
    xp = np.zeros((B, 257, C, WT), dtype=ml_dtypes.bfloat16)
    xp[:, :256, :, PAD : PAD + W] = np.transpose(x, (0, 2, 1, 3))
    in_maps = []
    for core in range(8):
        b, half = divmod(core, 2)
        r0 = half * POUT
        in_maps.append({"xin": np.ascontiguousarray(xp[b, r0 : r0 + ROWS_IN + 1])})
    return in_maps


def _gather(results):
    out = np.empty((B, 254, 254), dtype=np.float32)
    for core in range(8):
        b, half = divmod(core, 2)
        out[b, half * POUT : half * POUT + POUT, :] = results[core]["yout"]
    return out


def kernel(x, **_unused):
    nc = _get_program()
    res = run_bass_kernel_spmd(nc, _shard_inputs(x), core_ids=list(range(8)))
    return _gather(res.results)


def kernel_traced(x):
    """Same as kernel() but returns (output, BassKernelResults) with trace."""
    nc = _get_program()
    res = run_bass_kernel_spmd(
        nc, _shard_inputs(x), core_ids=list(range(8)), trace=True
    )
    return _gather(res.results), res


# revision 27
# speedup vs baseline: 1.4465x; 1.4465x over previous
"""Joint-entropy (KDE logsumexp over 3x3 windows) Trainium2 kernel, v5.

Math: for each 3x3 window of pixel vectors v_n (C=3 channels),
  out[i,j] = log_norm - (1/9) * sum_n log(S_n),  S_n = sum_m exp(-2*||v_n-v_m||^2)

Per-pair Gaussians via Act's Derivative_Erf: derf(sqrt(2)*d) =
(2/sqrt(pi))*exp(-2 d^2), so prod_c derf = k*exp(-2||d||^2), k=(2/sqrt(pi))^3.
Every E value carries k; the self term "+1" becomes "+k" (folded into the
same-row A combos) and the final affine adds ln(k):
  out = (log_norm + ln k) - (1/9) * ln prod_n (k + sum_{m!=n} kE_nm).

Sharding: 8 cores = 4 batches x 2 row-halves; host-prepped bf16 slab
[130, 3, 264] per core (129 rows + 1 pad row, width padded 4 each side);
output [127, 254] f32. partitions = image rows; X[p, s] = row p+s (fused
overlapping DMA reads) makes all row gaps reachable with partition-0
operands. Input lands as two DMAs (s=0,1 then s=2) so compute starts as
soon as the first two views arrive.

E classes (sub -> Derivative_Erf -> 2 channel-muls), anchor cols -2..257:
  M1 [128,5,260] a=1 (X0 vs X1),  M2 [128,5,260] a=2 (X0 vs X2; row 127
  junk from the pad row, never consumed),  ME [128,2(lo,hi),2,260] a=0.

Stage C: sliding 3-sums over b (D = down pairs, U = up pairs with
per-plane col shifts via plane-stride tricks), A combos (+k folded, merged
over lo/hi). Y = Ahi + U1 gives roles S1 = Y + D1@+1 and S2 = Y@+1 + U2.
The two partition shifts go through DRAM round-trips (SBUF->DRAM->SBUF on
the two HWDGE queues): direct SBUF->SBUF DMA measured ~6x slower than the
DRAM bounce, which fans out over all 16 DMA engines.
Stage D: 4 muls, Ln, affine; output split across both HWDGE queues
(keeps the slow SWDGE drain off the exit path).
"""

import dataclasses

import numpy as np
import ml_dtypes

import concourse.bacc as bacc
import concourse.tile as tile
from concourse import mybir
from concourse.bass_utils import run_bass_kernel_spmd

F32 = mybir.dt.float32
BF16 = mybir.dt.bfloat16
AOP = mybir.AluOpType
AF = mybir.ActivationFunctionType

B = 4
C = 3
W = 256
PAD = 4           # host zero-pad each side
WT = W + 2 * PAD  # 264
WA = W + 4        # 260: anchor cols -2..257
ROWS_IN = 129
P = 128
POUT = 127
WOUT = 254
SROW = C * WT     # one input row in elements (792)
SQRT2 = float(np.sqrt(2.0))
CDERF = float(2.0 / np.sqrt(np.pi))
K = CDERF**3
LOG_NORM = float(np.log(9.0) + 3.0 * np.log(np.sqrt(2.0 * np.pi) * 0.5))
AFFINE_C = LOG_NORM + 3.0 * float(np.log(CDERF))


def _with_dims(base_ap, dims):
    """Replace free dims of `base_ap` (partition dim kept) with the given
    [stride, count] pairs (strides in elements)."""
    ap = [list(base_ap.ap[0])] + [list(d) for d in dims]
    return dataclasses.replace(base_ap, ap=ap)


def _build_program():
    nc = bacc.Bacc("TRN2")
    xin = nc.dram_tensor("xin", (ROWS_IN + 1, C, WT), BF16, kind="ExternalInput")
    wsh = nc.dram_tensor("wshift", (P, P), BF16, kind="ExternalInput")
    yout = nc.dram_tensor("yout", (POUT, WOUT), F32, kind="ExternalOutput")

    with tile.TileContext(nc) as tc:
        with (
            tc.tile_pool(name="p", bufs=1) as tp,
            tc.tile_pool(name="pp", bufs=1, space="PSUM") as pp,
        ):
            def ap_of(base, elem_off, dims):
                return dataclasses.replace(
                    _with_dims(base, dims), offset=base.offset + elem_off
                )

            # ---- load X[p, s] = input row p+s (s=0,1 first, then s=2) ---
            X = tp.tile([P, 3, C, WT], BF16, tag="x")
            src01 = _with_dims(xin[0:P, :, :], [[SROW, 2], [WT, C], [1, WT]])
            nc.gpsimd.dma_start(out=X[:, 0:2], in_=src01)
            src2 = ap_of(xin[0:P, :, :], 2 * SROW, [[WT, C], [1, WT]])
            nc.gpsimd.dma_start(out=X[:, 2], in_=src2)
            WS = tp.tile([P, P], BF16, tag="ws")
            nc.sync.dma_start(out=WS, in_=wsh[:, :])

            ME = tp.tile([P, 2, 2, WA], BF16, tag="me")  # a=0: s -> lo, hi

            def cls(tag, s_a, s_b, nb, c0, out_ap):
                """E class: anchor X[:, s_a] bcast over b; other X[:, s_b]
                at col c0+b; writes k*exp(-2 d2) planes into out_ap."""
                xa = X[0:P, 0, 0, 2 : 2 + WA]
                anchor = ap_of(xa, SROW * s_a, [[0, nb], [WT, C], [1, WA]])
                xb = X[0:P, 0, 0, c0 : c0 + WA]
                other = ap_of(xb, SROW * s_b, [[1, nb], [WT, C], [1, WA]])
                d = tp.tile([P, nb, C, WA], BF16, tag=f"d_{tag}")
                nc.vector.tensor_sub(d, anchor, other)
                g = tp.tile([P, nb, C, WA], BF16, tag=f"g_{tag}")
                nc.scalar.activation(g, d, AF.Derivative_Erf, scale=SQRT2)
                g01 = tp.tile([P, nb, WA], BF16, tag=f"g01_{tag}")
                nc.vector.tensor_mul(g01, g[:, :, 0, :], g[:, :, 1, :])
                nc.vector.tensor_mul(out_ap, g01, g[:, :, 2, :])

            def d_combo(mt, tag):
                t4 = tp.tile([P, 4, W], BF16, tag=f"t4{tag}")
                nc.vector.tensor_add(t4, mt[:, 0:4, 2 : 2 + W], mt[:, 1:5, 2 : 2 + W])
                out = tp.tile([P, 3, W], BF16, tag=f"dc{tag}")
                nc.vector.tensor_add(out, t4[:, 0:3, :], mt[:, 2:5, 2 : 2 + W])
                return out

            def u_combo(mt, tag):
                # plane t = sum_{j=t..t+2} mt[:, j, col + 4 - j]
                t4 = tp.tile([P, 4, W], BF16, tag=f"u4{tag}")
                in0 = ap_of(mt[0:P, 0, 0:W], 4, [[WA - 1, 4], [1, W]])
                in1 = ap_of(mt[0:P, 0, 0:W], WA + 3, [[WA - 1, 4], [1, W]])
                nc.vector.tensor_add(t4, in0, in1)
                out = tp.tile([P, 3, W], BF16, tag=f"uc{tag}")
                in2 = ap_of(mt[0:P, 0, 0:W], 2 * WA + 2, [[WA - 1, 3], [1, W]])
                nc.vector.tensor_add(out, t4[:, 0:3, :], in2)
                return out

            # ---- a=1 class first: its combos feed the DRAM bounce -------
            M1 = tp.tile([P, 5, WA], BF16, tag="m1")
            cls("m1", 0, 1, 5, 0, M1)
            D1 = d_combo(M1, "d1")
            U1 = u_combo(M1, "u1")

            # Partition shift by 1 on the (otherwise idle) PE:
            # out[m, :] = sum_k WS[k, m] * in[k, :] with WS = eye(k=-1),
            # i.e. out[m] = in[m+1].  Output lands in PSUM as f32.
            def pshift(t, tag):
                o = pp.tile([P, C * W], F32, tag=f"sh{tag}")
                rhs0 = ap_of(t[0:P, 0, 0:W], 0, [[1, 512]])
                nc.tensor.matmul(o[:, 0:512], WS, rhs0, start=True, stop=True)
                rhs1 = ap_of(t[0:P, 0, 0:W], 512, [[1, 256]])
                nc.tensor.matmul(o[:, 512:768], WS, rhs1, start=True, stop=True)
                return o

            D1h = pshift(D1, "d1")

            # ---- a=0 classes, A combos, Y -------------------------------
            cls("e0hi", 1, 1, 2, 3, ME[:, 1])
            cls("e0lo", 0, 0, 2, 3, ME[:, 0])

            # A combos (+k), merged over (lo, hi):
            #  nc=0: E01(j)+E02(j); nc=1: E01(j-1)+E01(j); nc=2: E02(j-2)+E01(j-1)
            SE = 2 * WA  # s stride of ME

            def me_ap(k, col0):
                return ap_of(ME[0:P, 0, 0, 0:W], WA * k + col0, [[SE, 2], [1, W]])

            A = tp.tile([P, 2, 3, W], BF16, tag="a")
            pairs = [((0, 2), (1, 2)), ((0, 1), (0, 2)), ((1, 0), (0, 1))]
            for ncol, ((k0, o0), (k1, o1)) in enumerate(pairs):
                nc.vector.scalar_tensor_tensor(
                    out=A[:, :, ncol, :],
                    in0=me_ap(k0, o0),
                    scalar=K,
                    in1=me_ap(k1, o1),
                    op0=AOP.add,
                    op1=AOP.add,
                )

            Y = tp.tile([P, 3, W], BF16, tag="y")
            nc.vector.tensor_add(Y, A[:, 1, :, :], U1)

            # ---- a=2 class + combos; Y bounce on the Act queue ----------
            M2 = tp.tile([P, 5, WA], BF16, tag="m2")
            cls("m2", 0, 2, 5, 0, M2)  # row 127 junk (pad row), not consumed

            Yh = pshift(Y, "y")

            D2 = d_combo(M2, "d2")
            U2 = u_combo(M2, "u2")

            # ---- role sums [127, 3(nc), 254] ----------------------------
            def diag(base, plane0, dplane):
                """[POUT, 3, WOUT] view: plane i at (plane0 + i*dplane, col+i);
                base = [POUT, WOUT] slice of a W-plane-stride tile."""
                return ap_of(base, plane0 * W, [[dplane * W + 1, 3], [1, WOUT]])

            S0 = tp.tile([POUT, 3, WOUT], BF16, tag="s0")
            nc.vector.tensor_add(
                S0, diag(A[0:POUT, 0, 0, 0:WOUT], 0, 1),
                diag(D1[0:POUT, 0, 0:WOUT], 2, -1),
            )
            nc.vector.tensor_add(S0, S0, diag(D2[0:POUT, 0, 0:WOUT], 2, -1))
            S1 = tp.tile([POUT, 3, WOUT], BF16, tag="s1")
            nc.vector.tensor_add(
                S1, diag(Y[0:POUT, 0, 0:WOUT], 0, 1),
                diag(D1h[0:POUT, 0:WOUT], 2, -1),
            )
            S2 = tp.tile([POUT, 3, WOUT], BF16, tag="s2")
            nc.vector.tensor_add(
                S2, diag(Yh[0:POUT, 0:WOUT], 0, 1),
                diag(U2[0:POUT, 0, 0:WOUT], 0, 1),
            )

            # ---- stage D: product, log, affine --------------------------
            T0 = tp.tile([POUT, 3, WOUT], BF16, tag="t0")
            nc.vector.tensor_mul(T0, S0, S1)
            T1 = tp.tile([POUT, 3, WOUT], BF16, tag="t1")
            nc.vector.tensor_mul(T1, T0, S2)
            R = tp.tile([POUT, WOUT], BF16, tag="r")
            nc.vector.tensor_mul(R, T1[:, 0, :], T1[:, 1, :])
            PP = tp.tile([POUT, WOUT], BF16, tag="pp")
            nc.vector.tensor_mul(PP, R, T1[:, 2, :])
            L = tp.tile([POUT, WOUT], F32, tag="lnp")
            nc.scalar.activation(L, PP, AF.Ln)
            OUT = tp.tile([POUT, WOUT], F32, tag="out")
            nc.vector.tensor_scalar(
                out=OUT,
                in0=L,
                scalar1=-1.0 / 9.0,
                scalar2=AFFINE_C,
                op0=AOP.mult,
                op1=AOP.add,
            )
            nc.sync.dma_start(out=yout[0:64, :], in_=OUT[0:64, :])
            nc.scalar.dma_start(out=yout[64:POUT, :], in_=OUT[64:POUT, :])
    if not nc.is_finalized():
        nc.finalize()
    return nc


_PROGRAM = None


def _get_program():
    global _PROGRAM
    if _PROGRAM is None:
        _PROGRAM = _build_program()
    return _PROGRAM


def _shard_inputs(x):
    x = np.asarray(x, dtype=np.float32)
    xp = np.zeros((B, 257, C, WT), dtype=ml_dtypes.bfloat16)
    xp[:, :256, :, PAD : PAD + W] = np.transpose(x, (0, 2, 1, 3))
    ws = np.eye(P, k=-1, dtype=ml_dtypes.bfloat16)
    in_maps = []
    for core in range(8):
        b, half = divmod(core, 2)
        r0 = half * POUT
        in_maps.append(
            {"xin": np.ascontiguousarray(xp[b, r0 : r0 + ROWS_IN + 1]), "wshift": ws}
        )
    return in_maps


def _gather(results):
    out = np.empty((B, 254, 254), dtype=np.float32)
    for core in range(8):
        b, half = divmod(core, 2)
        out[b, half * POUT : half * POUT + POUT, :] = results[core]["yout"]
    return out


def kernel(x, **_unused):
    nc = _get_program()
    res = run_bass_kernel_spmd(nc, _shard_inputs(x), core_ids=list(range(8)))
    return _gather(res.results)


def kernel_traced(x):
    """Same as kernel() but returns (output, BassKernelResults) with trace."""
    nc = _get_program()
    res = run_bass_kernel_spmd(
        nc, _shard_inputs(x), core_ids=list(range(8)), trace=True
    )
    return _gather(res.results), res


# revision 33
# speedup vs baseline: 1.5127x; 1.0458x over previous
"""Joint-entropy (KDE logsumexp over 3x3 windows) Trainium2 kernel, v5.

Math: for each 3x3 window of pixel vectors v_n (C=3 channels),
  out[i,j] = log_norm - (1/9) * sum_n log(S_n),  S_n = sum_m exp(-2*||v_n-v_m||^2)

Per-pair Gaussians via Act's Derivative_Erf: derf(sqrt(2)*d) =
(2/sqrt(pi))*exp(-2 d^2), so prod_c derf = k*exp(-2||d||^2), k=(2/sqrt(pi))^3.
Every E value carries k; the self term "+1" becomes "+k" (folded into the
same-row A combos) and the final affine adds ln(k):
  out = (log_norm + ln k) - (1/9) * ln prod_n (k + sum_{m!=n} kE_nm).

Sharding: 8 cores = 4 batches x 2 row-halves; host-prepped bf16 slab
[130, 3, 264] per core (129 rows + 1 pad row, width padded 4 each side);
output [127, 254] f32. partitions = image rows; X[p, s] = row p+s (fused
overlapping DMA reads) makes all row gaps reachable with partition-0
operands. Input lands as two DMAs (s=0,1 then s=2) so compute starts as
soon as the first two views arrive.

E classes (sub -> Derivative_Erf -> 2 channel-muls), anchor cols -2..257:
  M1 [128,5,260] a=1 (X0 vs X1),  M2 [128,5,260] a=2 (X0 vs X2; row 127
  junk from the pad row, never consumed),  ME [128,2(lo,hi),2,260] a=0.

Stage C: sliding 3-sums over b (D = down pairs, U = up pairs with
per-plane col shifts via plane-stride tricks), A combos (+k folded, merged
over lo/hi). Y = Ahi + U1 gives roles S1 = Y + D1@+1 and S2 = Y@+1 + U2.
The two partition shifts go through DRAM round-trips (SBUF->DRAM->SBUF on
the two HWDGE queues): direct SBUF->SBUF DMA measured ~6x slower than the
DRAM bounce, which fans out over all 16 DMA engines.
Stage D: 4 muls, Ln, affine; output split across both HWDGE queues
(keeps the slow SWDGE drain off the exit path).
"""

import dataclasses

import numpy as np
import ml_dtypes

import concourse.bacc as bacc
import concourse.tile as tile
from concourse import mybir
from concourse.bass_utils import run_bass_kernel_spmd

F32 = mybir.dt.float32
BF16 = mybir.dt.bfloat16
AOP = mybir.AluOpType
AF = mybir.ActivationFunctionType

B = 4
C = 3
W = 256
PAD = 4           # host zero-pad each side
WT = W + 2 * PAD  # 264
WA = W + 4        # 260: anchor cols -2..257
ROWS_IN = 129
P = 128
POUT = 127
WOUT = 254
SROW = C * WT     # one input row in elements (792)
SQRT2 = float(np.sqrt(2.0))
CDERF = float(2.0 / np.sqrt(np.pi))
K = CDERF**3
LOG_NORM = float(np.log(9.0) + 3.0 * np.log(np.sqrt(2.0 * np.pi) * 0.5))
AFFINE_C = LOG_NORM + 3.0 * float(np.log(CDERF))


def _with_dims(base_ap, dims):
    """Replace free dims of `base_ap` (partition dim kept) with the given
    [stride, count] pairs (strides in elements)."""
    ap = [list(base_ap.ap[0])] + [list(d) for d in dims]
    return dataclasses.replace(base_ap, ap=ap)


def _build_program():
    nc = bacc.Bacc("TRN2")
    xin = nc.dram_tensor("xin", (ROWS_IN + 1, C, WT), BF16, kind="ExternalInput")
    wsh = nc.dram_tensor("wshift", (P, P), BF16, kind="ExternalInput")
    yout = nc.dram_tensor("yout", (POUT, WOUT), F32, kind="ExternalOutput")

    with tile.TileContext(nc) as tc:
        with (
            tc.tile_pool(name="p", bufs=1) as tp,
            tc.tile_pool(name="pp", bufs=1, space="PSUM") as pp,
        ):
            def ap_of(base, elem_off, dims):
                return dataclasses.replace(
                    _with_dims(base, dims), offset=base.offset + elem_off
                )

            # ---- load X01[p, s] = input row p+s (s=0,1), X2[p] = row p+2 --
            X01 = tp.tile([P, 2, C, WT], BF16, tag="x01")
            src01 = _with_dims(xin[0:P, :, :], [[SROW, 2], [WT, C], [1, WT]])
            nc.gpsimd.dma_start(out=X01, in_=src01)
            X2 = tp.tile([P, C, WT], BF16, tag="x2")
            src2 = ap_of(xin[0:P, :, :], 2 * SROW, [[WT, C], [1, WT]])
            nc.gpsimd.dma_start(out=X2, in_=src2)
            WS = tp.tile([P, P], BF16, tag="ws")
            nc.sync.dma_start(out=WS, in_=wsh[:, :])

            ME0 = tp.tile([P, 2, WA], BF16, tag="me0")  # a=0 rows 0..127
            ME1 = tp.tile([P, 2, WA], BF16, tag="me1")  # a=0 rows 1..128

            def cls(tag, xa, xb, nb, out_ap):
                """E class: anchor `xa` bcast over b; other `xb` at col +b;
                both [P, C, *]-shaped slice APs. Writes k*exp(-2 d2) planes."""
                anchor = _with_dims(xa, [[0, nb], [WT, C], [1, WA]])
                other = _with_dims(xb, [[1, nb], [WT, C], [1, WA]])
                d = tp.tile([P, nb, C, WA], BF16, tag=f"d_{tag}")
                nc.vector.tensor_sub(d, anchor, other)
                g = tp.tile([P, nb, C, WA], BF16, tag=f"g_{tag}")
                nc.scalar.activation(g, d, AF.Derivative_Erf, scale=SQRT2)
                g01 = tp.tile([P, nb, WA], BF16, tag=f"g01_{tag}")
                nc.vector.tensor_mul(g01, g[:, :, 0, :], g[:, :, 1, :])
                nc.vector.tensor_mul(out_ap, g01, g[:, :, 2, :])

            def d_combo(mt, tag):
                t4 = tp.tile([P, 4, W], BF16, tag=f"t4{tag}")
                nc.vector.tensor_add(t4, mt[:, 0:4, 2 : 2 + W], mt[:, 1:5, 2 : 2 + W])
                out = tp.tile([P, 3, W], BF16, tag=f"dc{tag}")
                nc.vector.tensor_add(out, t4[:, 0:3, :], mt[:, 2:5, 2 : 2 + W])
                return out

            def u_combo(mt, tag):
                # plane t = sum_{j=t..t+2} mt[:, j, col + 4 - j]
                t4 = tp.tile([P, 4, W], BF16, tag=f"u4{tag}")
                in0 = ap_of(mt[0:P, 0, 0:W], 4, [[WA - 1, 4], [1, W]])
                in1 = ap_of(mt[0:P, 0, 0:W], WA + 3, [[WA - 1, 4], [1, W]])
                nc.vector.tensor_add(t4, in0, in1)
                out = tp.tile([P, 3, W], BF16, tag=f"uc{tag}")
                in2 = ap_of(mt[0:P, 0, 0:W], 2 * WA + 2, [[WA - 1, 3], [1, W]])
                nc.vector.tensor_add(out, t4[:, 0:3, :], in2)
                return out

            # ---- a=1 class first: its combos feed the PE shift ----------
            M1 = tp.tile([P, 5, WA], BF16, tag="m1")
            cls("m1", X01[0:P, 0, :, 2 : 2 + WA], X01[0:P, 1, :, 0:WA], 5, M1)
            D1 = d_combo(M1, "d1")
            U1 = u_combo(M1, "u1")

            # Partition shift by 1 on the (otherwise idle) PE:
            # out[m, :] = sum_k WS[k, m] * in[k, :] with WS = eye(k=-1),
            # i.e. out[m] = in[m+1].  Output lands in PSUM as f32.
            def pshift(t, tag):
                o = pp.tile([P, C * W], F32, tag=f"sh{tag}")
                rhs0 = ap_of(t[0:P, 0, 0:W], 0, [[1, 512]])
                nc.tensor.matmul(o[:, 0:512], WS, rhs0, start=True, stop=True)
                rhs1 = ap_of(t[0:P, 0, 0:W], 512, [[1, 256]])
                nc.tensor.matmul(o[:, 512:768], WS, rhs1, start=True, stop=True)
                return o

            D1h = pshift(D1, "d1")

            # ---- a=0 classes, A combos, Y -------------------------------
            cls("e0hi", X01[0:P, 1, :, 2 : 2 + WA], X01[0:P, 1, :, 3 : 3 + WA],
                2, ME1)
            cls("e0lo", X01[0:P, 0, :, 2 : 2 + WA], X01[0:P, 0, :, 3 : 3 + WA],
                2, ME0)

            # A combos per row alignment (pair sums of a=0 E maps):
            #  nc=0: E01(j)+E02(j); nc=1: E01(j-1)+E01(j); nc=2: E02(j-2)+E01(j-1)
            # The self term +k rides Alo only (S1/S2 get it via the PSUM
            # copy bias); Ahi stays on the fast tensor_add path.
            pairs = [((0, 2), (1, 2)), ((0, 1), (0, 2)), ((1, 0), (0, 1))]

            Ahi = tp.tile([P, 3, W], BF16, tag="ahi")
            for ncol, ((k0, o0), (k1, o1)) in enumerate(pairs):
                nc.vector.tensor_add(
                    Ahi[:, ncol, :],
                    ME1[0:P, k0, o0 : o0 + W],
                    ME1[0:P, k1, o1 : o1 + W],
                )

            Y = tp.tile([P, 3, W], BF16, tag="y")
            nc.vector.tensor_add(Y, Ahi, U1)

            # ---- a=2 class + combos; Y shift on PE ----------------------
            M2 = tp.tile([P, 5, WA], BF16, tag="m2")
            cls("m2", X01[0:P, 0, :, 2 : 2 + WA], X2[0:P, :, 0:WA], 5, M2)

            Yh = pshift(Y, "y")

            Alo = tp.tile([P, 3, W], BF16, tag="alo")
            for ncol, ((k0, o0), (k1, o1)) in enumerate(pairs):
                nc.vector.scalar_tensor_tensor(
                    out=Alo[:, ncol, :],
                    in0=ME0[0:P, k0, o0 : o0 + W],
                    scalar=K,
                    in1=ME0[0:P, k1, o1 : o1 + W],
                    op0=AOP.add,
                    op1=AOP.add,
                )

            D2 = d_combo(M2, "d2")
            U2 = u_combo(M2, "u2")

            # PSUM -> SBUF copies on the idle Act engine; bias adds the +k
            # self term for S1/S2 and the cast restores 2x DVE reads.
            KB = tp.tile([P, 1], F32, tag="kb")
            nc.gpsimd.memset(KB, K)
            D1c = tp.tile([P, C * W], BF16, tag="d1c")
            nc.scalar.activation(D1c, D1h, AF.Identity, bias=KB[:, :])
            Yc = tp.tile([P, C * W], BF16, tag="yc")
            nc.scalar.activation(Yc, Yh, AF.Identity, bias=KB[:, :])

            # ---- role sums [127, 3(nc), 254] ----------------------------
            def diag(base, plane0, dplane):
                """[POUT, 3, WOUT] view: plane i at (plane0 + i*dplane, col+i);
                base = [POUT, WOUT] slice of a W-plane-stride tile."""
                return ap_of(base, plane0 * W, [[dplane * W + 1, 3], [1, WOUT]])

            S0 = tp.tile([POUT, 3, WOUT], BF16, tag="s0")
            nc.vector.tensor_add(
                S0, diag(Alo[0:POUT, 0, 0:WOUT], 0, 1),
                diag(D1[0:POUT, 0, 0:WOUT], 2, -1),
            )
            nc.vector.tensor_add(S0, S0, diag(D2[0:POUT, 0, 0:WOUT], 2, -1))
            S1 = tp.tile([POUT, 3, WOUT], BF16, tag="s1")
            nc.vector.tensor_add(
                S1, diag(Y[0:POUT, 0, 0:WOUT], 0, 1),
                diag(D1c[0:POUT, 0:WOUT], 2, -1),
            )
            S2 = tp.tile([POUT, 3, WOUT], BF16, tag="s2")
            nc.vector.tensor_add(
                S2, diag(Yc[0:POUT, 0:WOUT], 0, 1),
                diag(U2[0:POUT, 0, 0:WOUT], 0, 1),
            )

            # ---- stage D: product, log, affine --------------------------
            T0 = tp.tile([POUT, 3, WOUT], BF16, tag="t0")
            nc.vector.tensor_mul(T0, S0, S1)
            T1 = tp.tile([POUT, 3, WOUT], BF16, tag="t1")
            nc.vector.tensor_mul(T1, T0, S2)
            R = tp.tile([POUT, WOUT], BF16, tag="r")
            nc.vector.tensor_mul(R, T1[:, 0, :], T1[:, 1, :])
            PP = tp.tile([POUT, WOUT], BF16, tag="pp")
            nc.vector.tensor_mul(PP, R, T1[:, 2, :])
            L = tp.tile([POUT, WOUT], F32, tag="lnp")
            nc.scalar.activation(L, PP, AF.Ln)
            OUT = tp.tile([POUT, WOUT], F32, tag="out")
            nc.vector.tensor_scalar(
                out=OUT,
                in0=L,
                scalar1=-1.0 / 9.0,
                scalar2=AFFINE_C,
                op0=AOP.mult,
                op1=AOP.add,
            )
            nc.sync.dma_start(out=yout[0:64, :], in_=OUT[0:64, :])
            nc.scalar.dma_start(out=yout[64:POUT, :], in_=OUT[64:POUT, :])
    if not nc.is_finalized():
        nc.finalize()
    return nc


_PROGRAM = None


def _get_program():
    global _PROGRAM
    if _PROGRAM is None:
        _PROGRAM = _build_program()
    return _PROGRAM


def _shard_inputs(x):
    x = np.asarray(x, dtype=np.float32)
    xp = np.zeros((B, 257, C, WT), dtype=ml_dtypes.bfloat16)
    xp[:, :256, :, PAD : PAD + W] = np.transpose(x, (0, 2, 1, 3))
    ws = np.eye(P, k=-1, dtype=ml_dtypes.bfloat16)
    in_maps = []
    for core in range(8):
        b, half = divmod(core, 2)
        r0 = half * POUT
        in_maps.append(
            {"xin": np.ascontiguousarray(xp[b, r0 : r0 + ROWS_IN + 1]), "wshift": ws}
        )
    return in_maps


def _gather(results):
    out = np.empty((B, 254, 254), dtype=np.float32)
    for core in range(8):
        b, half = divmod(core, 2)
        out[b, half * POUT : half * POUT + POUT, :] = results[core]["yout"]
    return out


def kernel(x, **_unused):
    nc = _get_program()
    res = run_bass_kernel_spmd(nc, _shard_inputs(x), core_ids=list(range(8)))
    return _gather(res.results)


def kernel_traced(x):
    """Same as kernel() but returns (output, BassKernelResults) with trace."""
    nc = _get_program()
    res = run_bass_kernel_spmd(
        nc, _shard_inputs(x), core_ids=list(range(8)), trace=True
    )
    return _gather(res.results), res


# revision 38
# speedup vs baseline: 1.5554x; 1.0282x over previous
"""Joint-entropy (KDE logsumexp over 3x3 windows) Trainium2 kernel, v5.

Math: for each 3x3 window of pixel vectors v_n (C=3 channels),
  out[i,j] = log_norm - (1/9) * sum_n log(S_n),  S_n = sum_m exp(-2*||v_n-v_m||^2)

Per-pair Gaussians via Act's Derivative_Erf: derf(sqrt(2)*d) =
(2/sqrt(pi))*exp(-2 d^2), so prod_c derf = k*exp(-2||d||^2), k=(2/sqrt(pi))^3.
Every E value carries k; the self term "+1" becomes "+k" (folded into the
same-row A combos) and the final affine adds ln(k):
  out = (log_norm + ln k) - (1/9) * ln prod_n (k + sum_{m!=n} kE_nm).

Sharding: 8 cores = 4 batches x 2 row-halves; host-prepped bf16 slab
[130, 3, 264] per core (129 rows + 1 pad row, width padded 4 each side);
output [127, 254] f32. partitions = image rows; X[p, s] = row p+s (fused
overlapping DMA reads) makes all row gaps reachable with partition-0
operands. Input lands as two DMAs (s=0,1 then s=2) so compute starts as
soon as the first two views arrive.

E classes (sub -> Derivative_Erf -> 2 channel-muls), anchor cols -2..257:
  M1 [128,5,260] a=1 (X0 vs X1),  M2 [128,5,260] a=2 (X0 vs X2; row 127
  junk from the pad row, never consumed),  ME [128,2(lo,hi),2,260] a=0.

Stage C: sliding 3-sums over b (D = down pairs, U = up pairs with
per-plane col shifts via plane-stride tricks), A combos (+k folded, merged
over lo/hi). Y = Ahi + U1 gives roles S1 = Y + D1@+1 and S2 = Y@+1 + U2.
The two partition shifts go through DRAM round-trips (SBUF->DRAM->SBUF on
the two HWDGE queues): direct SBUF->SBUF DMA measured ~6x slower than the
DRAM bounce, which fans out over all 16 DMA engines.
Stage D: 4 muls, Ln, affine; output split across both HWDGE queues
(keeps the slow SWDGE drain off the exit path).
"""

import dataclasses

import numpy as np
import ml_dtypes

import concourse.bacc as bacc
import concourse.tile as tile
from concourse import mybir
from concourse.bass_utils import run_bass_kernel_spmd

F32 = mybir.dt.float32
BF16 = mybir.dt.bfloat16
AOP = mybir.AluOpType
AF = mybir.ActivationFunctionType

B = 4
C = 3
W = 256
PAD = 4           # host zero-pad each side
WT = W + 2 * PAD  # 264
WA = W + 4        # 260: anchor cols -2..257
ROWS_IN = 129
P = 128
POUT = 127
WOUT = 254
SROW = C * WT     # one input row in elements (792)
SQRT2 = float(np.sqrt(2.0))
CDERF = float(2.0 / np.sqrt(np.pi))
K = CDERF**3
LOG_NORM = float(np.log(9.0) + 3.0 * np.log(np.sqrt(2.0 * np.pi) * 0.5))
AFFINE_C = LOG_NORM + 3.0 * float(np.log(CDERF))


def _with_dims(base_ap, dims):
    """Replace free dims of `base_ap` (partition dim kept) with the given
    [stride, count] pairs (strides in elements)."""
    ap = [list(base_ap.ap[0])] + [list(d) for d in dims]
    return dataclasses.replace(base_ap, ap=ap)


def _build_program():
    nc = bacc.Bacc("TRN2")
    xin = nc.dram_tensor("xin", (ROWS_IN + 1, C, WT), BF16, kind="ExternalInput")
    wsh = nc.dram_tensor("wshift", (P, P), BF16, kind="ExternalInput")
    yout = nc.dram_tensor("yout", (POUT, WOUT), F32, kind="ExternalOutput")

    with tile.TileContext(nc) as tc:
        with (
            tc.tile_pool(name="p", bufs=1) as tp,
            tc.tile_pool(name="pp", bufs=1, space="PSUM") as pp,
        ):
            def ap_of(base, elem_off, dims):
                return dataclasses.replace(
                    _with_dims(base, dims), offset=base.offset + elem_off
                )

            # ---- load Xs[p] = input row p+s, one DMA per shift ----------
            XS = []
            for s in range(3):
                xt = tp.tile([P, C, WT], BF16, tag=f"x{s}")
                nc.gpsimd.dma_start(
                    out=xt, in_=ap_of(xin[0:P, :, :], s * SROW, [[WT, C], [1, WT]])
                )
                XS.append(xt)
            X0, X1, X2 = XS
            WS = tp.tile([P, P], BF16, tag="ws")
            nc.sync.dma_start(out=WS, in_=wsh[:, :])

            ME0 = tp.tile([P, 2, WA], BF16, tag="me0")  # a=0 rows 0..127
            ME1 = tp.tile([P, 2, WA], BF16, tag="me1")  # a=0 rows 1..128

            def cls(tag, xa, xb, nb, out_ap):
                """E class: anchor `xa` bcast over b; other `xb` at col +b;
                both [P, C, *]-shaped slice APs. Writes k*exp(-2 d2) planes."""
                anchor = _with_dims(xa, [[0, nb], [WT, C], [1, WA]])
                other = _with_dims(xb, [[1, nb], [WT, C], [1, WA]])
                d = tp.tile([P, nb, C, WA], BF16, tag=f"d_{tag}")
                nc.vector.tensor_sub(d, anchor, other)
                g = tp.tile([P, nb, C, WA], BF16, tag=f"g_{tag}")
                nc.scalar.activation(g, d, AF.Derivative_Erf, scale=SQRT2)
                g01 = tp.tile([P, nb, WA], BF16, tag=f"g01_{tag}")
                nc.vector.tensor_mul(g01, g[:, :, 0, :], g[:, :, 1, :])
                nc.vector.tensor_mul(out_ap, g01, g[:, :, 2, :])

            def d_combo(mt, tag):
                t4 = tp.tile([P, 4, W], BF16, tag=f"t4{tag}")
                nc.vector.tensor_add(t4, mt[:, 0:4, 2 : 2 + W], mt[:, 1:5, 2 : 2 + W])
                out = tp.tile([P, 3, W], BF16, tag=f"dc{tag}")
                nc.vector.tensor_add(out, t4[:, 0:3, :], mt[:, 2:5, 2 : 2 + W])
                return out

            def u_combo(mt, tag):
                # plane t = sum_{j=t..t+2} mt[:, j, col + 4 - j]
                t4 = tp.tile([P, 4, W], BF16, tag=f"u4{tag}")
                in0 = ap_of(mt[0:P, 0, 0:W], 4, [[WA - 1, 4], [1, W]])
                in1 = ap_of(mt[0:P, 0, 0:W], WA + 3, [[WA - 1, 4], [1, W]])
                nc.vector.tensor_add(t4, in0, in1)
                out = tp.tile([P, 3, W], BF16, tag=f"uc{tag}")
                in2 = ap_of(mt[0:P, 0, 0:W], 2 * WA + 2, [[WA - 1, 3], [1, W]])
                nc.vector.tensor_add(out, t4[:, 0:3, :], in2)
                return out

            # ---- e0lo first (only needs X0), then the a=1 class ---------
            cls("e0lo", X0[0:P, :, 2 : 2 + WA], X0[0:P, :, 3 : 3 + WA], 2, ME0)
            M1 = tp.tile([P, 5, WA], BF16, tag="m1")
            cls("m1", X0[0:P, :, 2 : 2 + WA], X1[0:P, :, 0:WA], 5, M1)
            D1 = d_combo(M1, "d1")
            U1 = u_combo(M1, "u1")

            # Partition shift by 1 on the (otherwise idle) PE:
            # out[m, :] = sum_k WS[k, m] * in[k, :] with WS = eye(k=-1),
            # i.e. out[m] = in[m+1].  Output lands in PSUM as f32.
            def pshift(t, tag):
                o = pp.tile([P, C * W], F32, tag=f"sh{tag}")
                rhs0 = ap_of(t[0:P, 0, 0:W], 0, [[1, 512]])
                nc.tensor.matmul(o[:, 0:512], WS, rhs0, start=True, stop=True)
                rhs1 = ap_of(t[0:P, 0, 0:W], 512, [[1, 256]])
                nc.tensor.matmul(o[:, 512:768], WS, rhs1, start=True, stop=True)
                return o

            D1h = pshift(D1, "d1")

            # ---- remaining a=0 class, A combos, Y -----------------------
            cls("e0hi", X1[0:P, :, 2 : 2 + WA], X1[0:P, :, 3 : 3 + WA], 2, ME1)

            # A combos per row alignment (pair sums of a=0 E maps):
            #  nc=0: E01(j)+E02(j); nc=1: E01(j-1)+E01(j); nc=2: E02(j-2)+E01(j-1)
            # The self term +k rides Alo only (S1/S2 get it via the PSUM
            # copy bias); Ahi stays on the fast tensor_add path.
            pairs = [((0, 2), (1, 2)), ((0, 1), (0, 2)), ((1, 0), (0, 1))]

            Ahi = tp.tile([P, 3, W], BF16, tag="ahi")
            for ncol, ((k0, o0), (k1, o1)) in enumerate(pairs):
                nc.vector.tensor_add(
                    Ahi[:, ncol, :],
                    ME1[0:P, k0, o0 : o0 + W],
                    ME1[0:P, k1, o1 : o1 + W],
                )

            Y = tp.tile([P, 3, W], BF16, tag="y")
            nc.vector.tensor_add(Y, Ahi, U1)

            # ---- a=2 class + combos; Y shift on PE ----------------------
            M2 = tp.tile([P, 5, WA], BF16, tag="m2")
            cls("m2", X0[0:P, :, 2 : 2 + WA], X2[0:P, :, 0:WA], 5, M2)

            Yh = pshift(Y, "y")

            Alo = tp.tile([P, 3, W], BF16, tag="alo")
            for ncol, ((k0, o0), (k1, o1)) in enumerate(pairs):
                nc.vector.scalar_tensor_tensor(
                    out=Alo[:, ncol, :],
                    in0=ME0[0:P, k0, o0 : o0 + W],
                    scalar=K,
                    in1=ME0[0:P, k1, o1 : o1 + W],
                    op0=AOP.add,
                    op1=AOP.add,
                )

            D2 = d_combo(M2, "d2")
            U2 = u_combo(M2, "u2")

            # PSUM -> SBUF copies on the idle Act engine; bias adds the +k
            # self term for S1/S2 and the cast restores 2x DVE reads.
            KB = tp.tile([P, 1], F32, tag="kb")
            nc.gpsimd.memset(KB, K)
            D1c = tp.tile([P, C * W], BF16, tag="d1c")
            nc.scalar.activation(D1c, D1h, AF.Identity, bias=KB[:, :])
            Yc = tp.tile([P, C * W], BF16, tag="yc")
            nc.scalar.activation(Yc, Yh, AF.Identity, bias=KB[:, :])

            # ---- role sums [127, 3(nc), 254] ----------------------------
            def diag(base, plane0, dplane):
                """[POUT, 3, WOUT] view: plane i at (plane0 + i*dplane, col+i);
                base = [POUT, WOUT] slice of a W-plane-stride tile."""
                return ap_of(base, plane0 * W, [[dplane * W + 1, 3], [1, WOUT]])

            S0 = tp.tile([POUT, 3, WOUT], BF16, tag="s0")
            nc.vector.tensor_add(
                S0, diag(Alo[0:POUT, 0, 0:WOUT], 0, 1),
                diag(D1[0:POUT, 0, 0:WOUT], 2, -1),
            )
            nc.vector.tensor_add(S0, S0, diag(D2[0:POUT, 0, 0:WOUT], 2, -1))
            S1 = tp.tile([POUT, 3, WOUT], BF16, tag="s1")
            nc.vector.tensor_add(
                S1, diag(Y[0:POUT, 0, 0:WOUT], 0, 1),
                diag(D1c[0:POUT, 0:WOUT], 2, -1),
            )
            S2 = tp.tile([POUT, 3, WOUT], BF16, tag="s2")
            nc.vector.tensor_add(
                S2, diag(Yc[0:POUT, 0:WOUT], 0, 1),
                diag(U2[0:POUT, 0, 0:WOUT], 0, 1),
            )

            # ---- stage D: product, log, affine --------------------------
            T0 = tp.tile([POUT, 3, WOUT], BF16, tag="t0")
            nc.vector.tensor_mul(T0, S0, S1)
            T1 = tp.tile([POUT, 3, WOUT], BF16, tag="t1")
            nc.vector.tensor_mul(T1, T0, S2)
            R = tp.tile([POUT, WOUT], BF16, tag="r")
            nc.vector.tensor_mul(R, T1[:, 0, :], T1[:, 1, :])
            PP = tp.tile([POUT, WOUT], BF16, tag="pp")
            nc.vector.tensor_mul(PP, R, T1[:, 2, :])
            L = tp.tile([POUT, WOUT], F32, tag="lnp")
            nc.scalar.activation(L, PP, AF.Ln)
            AB = tp.tile([P, 1], F32, tag="ab")
            nc.gpsimd.memset(AB, AFFINE_C)
            OUT = tp.tile([POUT, WOUT], F32, tag="out")
            nc.scalar.activation(
                OUT, L, AF.Identity, scale=-1.0 / 9.0, bias=AB[0:POUT, :]
            )
            nc.sync.dma_start(out=yout[0:64, :], in_=OUT[0:64, :])
            nc.scalar.dma_start(out=yout[64:POUT, :], in_=OUT[64:POUT, :])
    if not nc.is_finalized():
        nc.finalize()
    return nc


_PROGRAM = None


def _get_program():
    global _PROGRAM
    if _PROGRAM is None:
        _PROGRAM = _build_program()
    return _PROGRAM


def _shard_inputs(x):
    x = np.asarray(x, dtype=np.float32)
    xp = np.zeros((B, 257, C, WT), dtype=ml_dtypes.bfloat16)
    xp[:, :256, :, PAD : PAD + W] = np.transpose(x, (0, 2, 1, 3))
    ws = np.eye(P, k=-1, dtype=ml_dtypes.bfloat16)
    in_maps = []
    for core in range(8):
        b, half = divmod(core, 2)
        r0 = half * POUT
        in_maps.append(
            {"xin": np.ascontiguousarray(xp[b, r0 : r0 + ROWS_IN + 1]), "wshift": ws}
        )
    return in_maps


def _gather(results):
    out = np.empty((B, 254, 254), dtype=np.float32)
    for core in range(8):
        b, half = divmod(core, 2)
        out[b, half * POUT : half * POUT + POUT, :] = results[core]["yout"]
    return out


def kernel(x, **_unused):
    nc = _get_program()
    res = run_bass_kernel_spmd(nc, _shard_inputs(x), core_ids=list(range(8)))
    return _gather(res.results)


def kernel_traced(x):
    """Same as kernel() but returns (output, BassKernelResults) with trace."""
    nc = _get_program()
    res = run_bass_kernel_spmd(
        nc, _shard_inputs(x), core_ids=list(range(8)), trace=True
    )
    return _gather(res.results), res
